# revision 47
# baseline (speedup 1.0000x reference)
"""BEV pooling (LSS view transform) kernel for Trainium2, 8 NeuronCores.

Problem: x (B=4, D=118, H=32, W=88, C=80) camera frustum features are pooled
into a (B, C, 360, 360) BEV grid via voxel scatter-add (segment_sum).

Structure exploited (verified at runtime from the actual inputs):
  - camera->lidar transform maps pixel (u, v, depth d): lidar (x, y) depend
    only on (u=w, d); lidar z depends only on (v=h, d).  So the BEV voxel of a
    point is a function of (d, w) alone, and the z-range keep-mask a function
    of (d, h) alone.
  - Therefore:  pooled[vox(d,w)] += sum_h zmask(d,h) * x[d,h,w,:]
  - Within a d-row, voxel ids are monotone in w, so equal-voxel groups are
    consecutive runs in w.
  - All cameras face +x, so real voxels span < 2*32767 linear ids; the
    int16-indexed scatter covers them with S segment tensors.

Device kernel per core (core = one batch x one 44-column w-half; runs that
cross the w boundary give partial sums in each core's private output, which
the host adds). Fully manual semaphore pipeline (no TileContext):

  PE+DMA stream: x streamed as bf16 [128, 3520] tiles (4 d-slabs each, 8-slot
      ring); PE bf16 matmul with a block-diagonal 0/1 h-mask reduces over h
      into PSUM y[118, 44*80] fp32 in two phases (d<64, d>=64).
  DVE dedup: Hillis-Steele masked shift-adds along w give every run-start
      slot the full run sum. Levels are per-phase: the lo phase (near depth,
      long runs) dedups under the hi phase's streaming shadow; the hi phase
      (far depth) usually has maxrun==1 and needs no levels at all.
  Scatter: 44 indirect-DMA write-scatters, one per w column with [D, 1]
      offsets — the only offset form real SWDGE supports (2D offset tables
      and partition-base-64 APs pass CoreSim but hang on HW).  Non-run-start
      / out-of-range slots point at a trash row (grid row V, discarded by
      assemble) so no bounds_check register compare is needed in descgen
      (saves ~37ns/call).  Plain writes, NOT dma_scatter_add: RMW descriptors
      measured ~8x slower per 320B token on HW (~640ns vs ~77ns each).
      Measured per-call cost: ~1.6us descgen (118 offset rows) + ~0.4us
      dispatch gap, serial on the gpsimd engine.  Tried and reverted:
      splitting calls across d-phases (per-call fixed cost exceeds what
      hides under the stream) and bf16 y/grid (SWDGE descgen is ~330ns/call
      SLOWER for 2-byte elements, outweighing the 2x DVE dedup gain).
"""

import os
import sys

import numpy as np

sys.path.insert(0, "/opt/trn_rl_repo")

# ---- problem constants (hardcoded per spec) ----
B, D, H, W, C = 4, 118, 32, 88, 80
WS = W // 2  # per-core w-column span (cores shard on batch x w-half)
CH = C
NXX = NXY = 360
NZ = 1
V = NXX * NXY
DX = np.array([0.3, 0.3, 20.0], np.float32)
BX_LO = np.array([-54.0, -54.0, -10.0], np.float32)
N_CORES = 8
GROUPS = (D + 3) // 4  # 30 groups of <=4 d-slabs
DLO = 64  # lo phase covers d in [0, 64), hi phase [64, D)
SENTINEL = 1 << 22  # run-boundary marker used during host planning
XBUFS = 8  # x-tile ring slots

_NC_CACHE: dict = {}


def _host_coords(x, camera2lidar_rots, camera2lidar_trans, intrins, frustum):
    """Voxel int coords for every point, bit-identical to the reference
    (same jax ops on the cpu backend)."""
    import jax
    import jax.numpy as jnp

    cpu = jax.devices("cpu")[0]
    with jax.default_device(cpu):
        frustum = jnp.asarray(np.asarray(frustum))
        rots = jnp.asarray(np.asarray(camera2lidar_rots))
        trans = jnp.asarray(np.asarray(camera2lidar_trans))
        intr = jnp.asarray(np.asarray(intrins))
        pts = jnp.concatenate(
            [frustum[..., :2] * frustum[..., 2:3], frustum[..., 2:3]], axis=-1
        )
        combine = rots @ jnp.linalg.inv(intr)
        geom = (
            jnp.einsum("bij,dhwj->bdhwi", combine, pts)
            + trans[:, None, None, None, :]
        )
        coords = ((geom - jnp.asarray(BX_LO)) / jnp.asarray(DX)).astype(jnp.int32)
        coords = np.asarray(jax.device_get(coords))
    return coords  # (B, D, H, W, 3) int32


def _host_fallback(x, camera2lidar_rots, camera2lidar_trans, intrins, frustum):
    """Exact reference computation on host (jax cpu). Correct for arbitrary
    inputs; used only if the factorized structure doesn't hold."""
    import jax
    import jax.numpy as jnp

    cpu = jax.devices("cpu")[0]
    with jax.default_device(cpu):
        x = jnp.asarray(np.asarray(x))
        rots = jnp.asarray(np.asarray(camera2lidar_rots))
        trans = jnp.asarray(np.asarray(camera2lidar_trans))
        intr = jnp.asarray(np.asarray(intrins))
        frustum = jnp.asarray(np.asarray(frustum))
        b, d, h, w, c = x.shape
        pts = jnp.concatenate(
            [frustum[..., :2] * frustum[..., 2:3], frustum[..., 2:3]], axis=-1
        )
        combine = rots @ jnp.linalg.inv(intr)
        geom = (
            jnp.einsum("bij,dhwj->bdhwi", combine, pts)
            + trans[:, None, None, None, :]
        )
        feats = x.reshape(-1, c)
        coords = ((geom - jnp.asarray(BX_LO)) / jnp.asarray(DX)).astype(
            jnp.int32
        ).reshape(-1, 3)
        npts = feats.shape[0]
        batch_ix = jnp.repeat(jnp.arange(b, dtype=jnp.int32), npts // b)
        nx = jnp.array([NXX, NXY, NZ], jnp.int32)
        kept = jnp.all((coords >= 0) & (coords < nx), axis=-1)
        lin = ((batch_ix * NZ + coords[:, 2]) * NXX + coords[:, 0]) * NXY + coords[:, 1]
        nseg = b * NZ * NXX * NXY
        lin = jnp.where(kept, lin, nseg)
        pooled = jax.ops.segment_sum(feats, lin, num_segments=nseg + 1)[:-1]
        out = pooled.reshape(b, NZ, NXX, NXY, c).transpose(0, 1, 4, 2, 3)
        final = out.reshape(b, NZ * c, NXX, NXY)
        return np.asarray(jax.device_get(final))


def _phase_levels(firstw, inrw, run_id, p0, p1):
    """Hillis-Steele level count needed for partitions [p0, p1)."""
    import numpy as _np

    maxrun = 1
    nb, nh = firstw.shape[:2]
    for b in range(nb):
        for h in range(nh):
            sel = inrw[b, h, p0:p1]
            if not sel.any():
                continue
            _, cnt = _np.unique(run_id[b, h, p0:p1][sel], return_counts=True)
            if cnt.size:
                maxrun = max(maxrun, int(cnt.max()))
    if maxrun <= 1:
        return 0
    return int(np.ceil(np.log2(maxrun)))


def plan(coords):
    """Build per-batch mask/offset tables from int voxel coords.

    Returns None if the (d,w)/(d,h) factorization doesn't hold (caller then
    uses the host fallback), else a dict of per-batch planning tensors.
    """
    cx, cy, cz = coords[..., 0], coords[..., 1], coords[..., 2]
    if not (
        (cx == cx[:, :, :1, :]).all()
        and (cy == cy[:, :, :1, :]).all()
        and (cz == cz[:, :, :, :1]).all()
    ):
        return None

    vx = cx[:, :, 0, :].astype(np.int64)  # (B, D, W)
    vy = cy[:, :, 0, :].astype(np.int64)
    zk = cz[:, :, :, 0] == 0  # (B, D, H) keep mask

    inr = (vx >= 0) & (vx < NXX) & (vy >= 0) & (vy < NXY)
    slot_ids = np.arange(D * W, dtype=np.int64).reshape(1, D, W)
    SENT = 1 << 22
    vox = np.where(inr, vx * NXY + vy, SENT + slot_ids)  # unique sentinels

    # Per (batch, w-half) window: runs of equal vox along the LOCAL w axis.
    firstw = np.ones((B, 2, D, WS), bool)
    inrw = np.zeros((B, 2, D, WS), bool)
    voxw = np.zeros((B, 2, D, WS), np.int64)
    for h in range(2):
        vw = vox[:, :, h * WS : (h + 1) * WS]
        voxw[:, h] = vw
        inrw[:, h] = inr[:, :, h * WS : (h + 1) * WS]
        firstw[:, h, :, 1:] = vw[:, :, 1:] != vw[:, :, :-1]

    run_id = np.cumsum(firstw.reshape(B * 2, -1), axis=1).reshape(B, 2, D, WS)
    lv_lo = max(1, _phase_levels(firstw, inrw, run_id, 0, DLO))
    lv_hi = _phase_levels(firstw, inrw, run_id, DLO, D)

    # shift masks per phase: dm[b,h,k,d,w] = 1 if voxw[d,w] == voxw[d,w+2^k]
    def shift_masks(levels, p0, p1):
        dm = np.zeros((B, 2, max(levels, 1), p1 - p0, WS), np.float32)
        for k in range(levels):
            s = 1 << k
            if s < WS:
                dm[:, :, k, :, : WS - s] = (
                    voxw[:, :, p0:p1, s:] == voxw[:, :, p0:p1, :-s]
                ).astype(np.float32)
        return dm

    dm_lo = shift_masks(lv_lo, 0, DLO)
    dm_hi = shift_masks(lv_hi, DLO, D)

    # scatter plan: run-start in-range slots carry their voxel id, everything
    # else a sentinel skipped via bounds_check
    scat = firstw & inrw
    offs = np.where(scat, voxw, V).astype(np.int32)  # dead slots -> trash row V

    # safety: within one core's window a voxel must not be scattered from
    # two different runs (plain writes would clobber). Fall back if so.
    for b in range(B):
        for h in range(2):
            v = voxw[b, h][scat[b, h]]
            if len(v) != len(np.unique(v)):
                return None

    # PE h-mask, one block per 4-d group (bf16: 0/1 exact).
    hm = np.zeros((B, GROUPS, 128, 64), np.float32)
    zkf = zk.astype(np.float32)
    for g in range(GROUPS):
        base = 0 if g < 16 else DLO
        for j in range(min(4, D - 4 * g)):
            hm[:, g, 32 * j : 32 * j + H, 4 * g + j - base] = zkf[:, 4 * g + j, :]

    return {
        "lv_lo": lv_lo,
        "lv_hi": lv_hi,
        "hm": hm,  # (B, GROUPS, 128, 64) f32
        "dm_lo": dm_lo,  # (B, 2, lv_lo, 64, WS) f32
        "dm_hi": dm_hi,  # (B, 2, max(lv_hi,1), 54, WS) f32
        "offs": offs,  # (B, 2, D, WS) i32
    }


def build_nc(lv_lo, lv_hi):
    """Build the (single, SPMD) manual-semaphore Bass program."""
    from contextlib import ExitStack

    from concourse import bacc, bass, mybir

    bf16 = mybir.dt.bfloat16
    f32 = mybir.dt.float32
    i32 = mybir.dt.int32

    nc = bacc.Bacc(
        trn_type="TRN2",
        target_bir_lowering=False,
        debug=False,
        enable_asserts=False,
        num_devices=N_CORES,
        # each indirect scatter call walks up to 64*44 offset rows of SWDGE
        # m2s descriptors; give the dynamic ring headroom
        dynamic_dma_scratch_size=1 << 16,
    )
    WC = WS * CH  # 3520
    LVW = (lv_lo + max(lv_hi, 1)) * WS

    x_d = nc.dram_tensor("x_s", (D, H, WS, CH), bf16, kind="ExternalInput")
    hm_d = nc.dram_tensor("hm", (128, GROUPS * 64), bf16, kind="ExternalInput")
    dm_d = nc.dram_tensor("dm", (D, LVW), f32, kind="ExternalInput")
    off_d = nc.dram_tensor("offs", (D, WS), i32, kind="ExternalInput")
    grid = nc.dram_tensor("grid", (V + 1, CH), f32, kind="ExternalOutput")

    # raw SBUF tensors (manual addressing, no pools)
    xt = nc.alloc_sbuf_tensor("xt", [128, XBUFS * WC], bf16).ap()
    y_t = nc.alloc_sbuf_tensor("y_t", [128, WC], f32).ap()
    tmp = nc.alloc_sbuf_tensor("tmp", [128, WC], f32).ap()
    hm_t = nc.alloc_sbuf_tensor("hm_t", [128, GROUPS * 64], bf16).ap()
    dm_t = nc.alloc_sbuf_tensor("dm_t", [128, LVW], f32).ap()
    off_t = nc.alloc_sbuf_tensor("off_t", [128, WS], i32).ap()
    y_ps = nc.alloc_psum_tensor("y_ps", [128, WC], f32).ap()

    y3 = y_t.rearrange("p (w c) -> p w c", c=CH)
    t3 = tmp.rearrange("p (w c) -> p w c", c=CH)

    es = ExitStack()
    sem = lambda nm: es.enter_context(nc.semaphore(nm))  # noqa: E731
    tb_hm = sem("tb_hm")  # per-table load sems (completions can reorder)
    tb_dm = sem("tb_dm")
    tb_off = sem("tb_off")
    slot = [sem(f"slot{j}") for j in range(XBUFS)]  # x tile landed
    freed = [sem(f"freed{j}") for j in range(XBUFS)]  # x tile consumed
    dsem = sem("dsem")  # scatter DMA landed
    mmc = sem("mmc")  # last-group per-chunk matmul completion
    # DVE write-completion chain: DVE does NOT interlock same-engine RAW
    # (writes land asynchronously), so every DVE op increments `vv` on write
    # completion and dependents wait for its position in the chain.
    vv = sem("vv")

    # ---- SYNC engine: all input DMA (x tile 0 queued ahead of the tables
    # so the PE pipeline starts as early as possible) ----
    def load_tile(g):
        j = g % XBUFS
        nd = min(4, D - 4 * g)
        if g >= XBUFS:
            nc.sync.wait_ge(freed[j], g // XBUFS)
        nc.sync.dma_start(
            out=xt[: 32 * nd, j * WC : j * WC + WC],
            in_=x_d.ap()[4 * g : 4 * g + nd].rearrange("d h w c -> (d h) (w c)"),
        ).then_inc(slot[j], 16)

    # group 0 loads in two column halves so PE can start on half A while
    # half B and the tables are still in flight (slot[0] reaches 32 for g0)
    g0ap = x_d.ap()[0:4].rearrange("d h w c -> (d h) (w c)")
    HALF = WC // 2
    nc.sync.dma_start(out=xt[:128, :HALF], in_=g0ap[:, :HALF]).then_inc(
        slot[0], 16
    )
    nc.sync.dma_start(out=hm_t[:], in_=hm_d.ap()).then_inc(tb_hm, 16)
    nc.sync.dma_start(out=xt[:128, HALF:WC], in_=g0ap[:, HALF:]).then_inc(
        slot[0], 16
    )
    nc.sync.dma_start(out=dm_t[:D, :], in_=dm_d.ap()).then_inc(tb_dm, 16)
    nc.sync.dma_start(out=off_t[:D, :], in_=off_d.ap()).then_inc(tb_off, 16)
    for g in range(1, GROUPS):
        load_tile(g)

    # ---- PE engine: block-diagonal h-mask matmuls, 2 PSUM phases ----
    nc.tensor.wait_ge(tb_hm, 16)  # hm loaded
    for g in range(GROUPS):
        j = g % XBUFS
        nd = min(4, D - 4 * g)
        rows = 32 * nd
        m = DLO if g < 16 else D - DLO
        first = g in (0, 16)
        last = g in (15, GROUPS - 1)
        if g == 0:
            pass  # per-chunk waits below (two half-loads)
        elif j == 0:
            nc.tensor.wait_ge(slot[0], 16 * (g // XBUFS + 1) + 16)
        else:
            nc.tensor.wait_ge(slot[j], 16 * (g // XBUFS + 1))
        if g == 16:
            nc.tensor.wait_ge(vv, 1)  # lo PSUM copied out before reuse
        for ci, n0 in enumerate(range(0, WC, 512)):
            nn = min(512, WC - n0)
            if g == 0 and n0 == 0:
                nc.tensor.wait_ge(slot[0], 16)  # half A (chunks 0-2)
            elif g == 0 and n0 == 1536:
                nc.tensor.wait_ge(slot[0], 32)  # half B (chunks 3-6)
            inst = nc.tensor.matmul(
                out=y_ps[:m, n0 : n0 + nn],
                lhsT=hm_t[:rows, g * 64 : g * 64 + m],
                rhs=xt[:rows, j * WC + n0 : j * WC + n0 + nn],
                start=first,
                stop=last,
            )
            if g == GROUPS - 1 and ci < 6:
                inst.then_inc(mmc, 1)  # per-chunk completion for the hi copy
        inst.then_inc(freed[j], 1)

    # ---- DVE engine: PSUM copy-out + per-phase Hillis-Steele dedup ----
    vn = 0

    def chain(inst):
        nonlocal vn
        vn += 1
        inst.then_inc(vv, 1)
        return inst

    def dedup(p0, p1, levels, dmcol0):
        for k in range(levels):
            s = 1 << k
            if s >= WS:
                break
            wl = WS - s
            mask = dm_t[p0:p1, dmcol0 + k * WS : dmcol0 + k * WS + wl]
            nc.vector.wait_ge(vv, vn)
            chain(
                nc.vector.tensor_tensor(
                    out=t3[p0:p1, :wl, :],
                    in0=y3[p0:p1, s:WS, :],
                    in1=mask[:, :, None].to_broadcast([p1 - p0, wl, CH]),
                    op=mybir.AluOpType.mult,
                )
            )
            nc.vector.wait_ge(vv, vn)
            yield chain(
                nc.vector.tensor_tensor(
                    out=y3[p0:p1, :wl, :],
                    in0=y3[p0:p1, :wl, :],
                    in1=t3[p0:p1, :wl, :],
                    op=mybir.AluOpType.add,
                )
            )

    # phase completion is observed via the freed[] sem attached to the last
    # matmul of g=15 / g=29 (a trailing PE nop would retire at issue time,
    # before the PSUM writes complete — caught by the CoreSim race detector)
    MM_LO = (15 % XBUFS, 15 // XBUFS + 1)
    MM_HI = ((GROUPS - 1) % XBUFS, (GROUPS - 1) // XBUFS + 1)
    nc.vector.wait_ge(tb_dm, 16)  # dm loaded
    nc.vector.wait_ge(freed[MM_LO[0]], MM_LO[1])
    chain(nc.vector.tensor_copy(out=y_t[:DLO, :], in_=y_ps[:DLO, :]))
    for _ in dedup(0, DLO, lv_lo, 0):
        pass
    VN_LO = vn  # lo half of y_t final once vv reaches this

    hi_chunk_vn = []  # vv threshold after each hi-copy column chunk
    for ci, n0 in enumerate(range(0, WC, 512)):
        nn = min(512, WC - n0)
        if ci < 6:
            nc.vector.wait_ge(mmc, ci + 1)
        else:
            nc.vector.wait_ge(freed[MM_HI[0]], MM_HI[1])
        chain(
            nc.vector.tensor_copy(
                out=y_t[DLO:D, n0 : n0 + nn], in_=y_ps[: D - DLO, n0 : n0 + nn]
            )
        )
        hi_chunk_vn.append(vn)
    for _ in dedup(DLO, D, lv_hi, lv_lo * WS):
        pass
    VN_END = vn  # all of y_t final once vv reaches this
    if lv_hi > 0:
        hi_chunk_vn = [VN_END] * len(hi_chunk_vn)

    # ---- GpSimd engine: indirect write-scatter, one call per w column with
    # [D, 1] offsets (the only offset form real SWDGE supports — 2D offset
    # tables pass CoreSim but hang on HW). Sentinel offsets are skipped via
    # bounds_check; written voxels are disjoint so no inter-call waits. ----
    nc.gpsimd.wait_ge(tb_off, 16)  # offset table loaded
    nc.gpsimd.wait_ge(vv, VN_LO)  # lo rows final
    cur_wait = -1
    for w in range(WS):
        need = hi_chunk_vn[min(((w + 1) * CH - 1) // 512, len(hi_chunk_vn) - 1)]
        if need > cur_wait:
            nc.gpsimd.wait_ge(vv, need)  # hi-copy chunk covering col w done
            cur_wait = need
        nc.gpsimd.indirect_dma_start(
            out=grid.ap(),
            out_offset=bass.IndirectOffsetOnAxis(ap=off_t[:D, w : w + 1], axis=0),
            in_=y_t[:D, w * CH : (w + 1) * CH],
            in_offset=None,
            bounds_check=None,
            oob_is_err=False,
        ).then_inc(dsem, 16)
    nc.gpsimd.wait_ge(dsem, 16 * WS)

    nc.compile()
    es.close()
    return nc


def make_in_maps(x, p):
    """Per-core input dicts. Core i: batch i//2, w-half i%2."""
    import ml_dtypes

    x = np.asarray(x)
    lv_lo, lv_hi = p["lv_lo"], p["lv_hi"]
    LVW = (lv_lo + max(lv_hi, 1)) * WS
    in_maps = []
    for core in range(N_CORES):
        b, half = core // 2, core % 2
        dm = np.zeros((D, LVW), np.float32)
        dm[:DLO, : lv_lo * WS] = (
            p["dm_lo"][b, half].transpose(1, 0, 2).reshape(DLO, lv_lo * WS)
        )
        if lv_hi > 0:
            dm[DLO:D, lv_lo * WS :] = (
                p["dm_hi"][b, half].transpose(1, 0, 2).reshape(D - DLO, lv_hi * WS)
            )
        in_maps.append(
            {
                "x_s": np.ascontiguousarray(
                    x[b, :, :, half * WS : (half + 1) * WS, :]
                ).astype(ml_dtypes.bfloat16),
                "hm": np.ascontiguousarray(
                    p["hm"][b].transpose(1, 0, 2).reshape(128, GROUPS * 64)
                ).astype(ml_dtypes.bfloat16),
                "dm": np.ascontiguousarray(dm),
                "offs": np.ascontiguousarray(p["offs"][b, half]),
            }
        )
    return in_maps


def assemble(results, p):
    """Sum per-core grids (w-half pairs add) into (B, C, 360, 360)."""
    out = np.empty((B, C, NXX, NXY), np.float32)
    for b in range(B):
        g = results[2 * b]["grid"][:V, :C] + results[2 * b + 1]["grid"][:V, :C]
        out[b] = g.reshape(NXX, NXY, C).transpose(2, 0, 1)
    return out


def _install_ntff_shim():
    """Provide antenv.axon_hooks with an NTFF profile hook driven by ctypes
    into the axon PJRT .so. Only used when KERNEL_TRACE=1."""
    import contextlib
    import ctypes
    import types

    if "antenv.axon_hooks" in sys.modules:
        return
    so_path = "/opt/axon/libaxon_pjrt.so"
    if not os.path.exists(so_path):
        return
    lib = ctypes.CDLL(so_path)
    if not hasattr(lib, "axon_start_nrt_profile"):
        return
    lib.axon_start_nrt_profile.argtypes = [
        ctypes.POINTER(ctypes.c_int64),
        ctypes.c_size_t,
    ]
    lib.axon_start_nrt_profile.restype = ctypes.c_int64
    lib.axon_stop_nrt_profile.argtypes = [ctypes.c_char_p]
    lib.axon_stop_nrt_profile.restype = ctypes.c_int64

    @contextlib.contextmanager
    def _hook(output_dir, device_ids):
        import jax

        jax.devices()
        if device_ids:
            ids = (ctypes.c_int64 * len(device_ids))(*device_ids)
            rc = lib.axon_start_nrt_profile(ids, len(device_ids))
        else:
            rc = lib.axon_start_nrt_profile(None, 0)
        if rc != 0:
            raise RuntimeError(f"axon_start_nrt_profile rc={rc}")
        try:
            yield
        finally:
            n = lib.axon_stop_nrt_profile(str(output_dir).encode())
            print(f"ntff profile: {n} file(s) written to {output_dir}")

    mod = types.ModuleType("antenv.axon_hooks")
    mod.get_axon_ntff_profile_hook = lambda: _hook
    mod.set_axon_ntff_profile_hook = lambda h: None
    sys.modules["antenv.axon_hooks"] = mod


def kernel(**inputs):
    x = np.asarray(inputs["x"])
    coords = _host_coords(**inputs)
    p = plan(coords)
    if p is None:
        return _host_fallback(**inputs)

    key = (p["lv_lo"], p["lv_hi"])
    if key not in _NC_CACHE:
        _NC_CACHE[key] = build_nc(*key)
    nc = _NC_CACHE[key]

    from concourse.bass_utils import run_bass_kernel_spmd

    trace = bool(int(os.environ.get("KERNEL_TRACE", "0")))
    trace_cores = None
    if trace:
        tc_env = os.environ.get("KERNEL_TRACE_CORES", "0")
        trace_cores = [int(t) for t in tc_env.split(",") if t != ""]
        _install_ntff_shim()
    res = run_bass_kernel_spmd(
        nc,
        make_in_maps(x, p),
        core_ids=list(range(N_CORES)),
        trace=trace,
        trace_cores=trace_cores,
    )
    kernel.last_results = res
    if res.exec_time_ns is not None:
        print(f"HW exec time: {res.exec_time_ns} ns")
    return assemble([res.results[i] for i in range(N_CORES)], p)


kernel.last_results = None
